# revision 1
# baseline (speedup 1.0000x reference)
"""AugLUT Trainium2 kernel: per-batch random 20-knot LUT applied to x via
piecewise-linear interpolation.

Math: with t = 19*x and the per-batch normalized LUT y[0..19],
    f(t) = sum_{k=-1}^{18} D_k * clamp(t - k, 0, 1)
where D_{-1} = y_0 (clamp == 1 for all t >= 0) and D_k = y_{k+1} - y_k.
Pairs of consecutive terms fuse into ONE custom DVE instruction via
    clamp(e-1,0,1) = clamp(e,0,2) - clamp(e,0,1):
    pair(a) = (D_a - D_{a+1})*clamp(e,0,1) + D_{a+1}*clamp(e,0,2),  e = t - a
so the whole 20-term LUT costs 10 DVE instructions per tile (+1 ACT scale).

Sharding: pure data parallel — batch b -> NeuronCore b (8 cores), the tiny
LUT rides along as a [128, 20] per-partition-broadcast input per core.
"""

import sys

if "/opt/trn_rl_repo" not in sys.path:
    sys.path.insert(0, "/opt/trn_rl_repo")

import numpy as np

import concourse.bacc as bacc
import concourse.dve_ops as dve_ops_mod
import concourse.mybir as mybir
from concourse import bass_utils
from concourse.dve_ops import DveOp
from concourse.dve_spec import (
    C0,
    C1,
    C2,
    Latch,
    One,
    Spec,
    Src0,
    Src1,
    Zero,
    lower,
    maxx,
    minn,
    _has_src1,
)
from concourse.dve_uop import DveOpSpec
from concourse.tile import TileContext

N_BINS = 20
EPS = 1e-5
BATCH = 8
SPATIAL = (192, 192, 192)
N_ELEM = 192 * 192 * 192  # 7_077_888
P = 128
FREE = N_ELEM // P  # 55296
CHUNK = 2048
N_CHUNKS = FREE // CHUNK  # 27


# --------------------------------------------------------------------------
# Custom DVE op registration (runtime, self-signed sha)
# --------------------------------------------------------------------------
def _pair_body(with_acc: bool):
    e = Src0 - C2
    r = maxx(e, Zero)
    c1 = minn(r, One)
    p1 = c1 * C0
    c2 = minn(r, One + One)
    if with_acc:
        a = Src1 + p1
        p2 = c2 * Latch(maxx(C1, C1))
        return a + p2
    p2 = c2 * C1
    return p1 + p2


def _np_pair(in0, in1, s0, s1, imm2, with_acc):
    e = in0.astype(np.float32) - np.float32(imm2)
    c1 = np.minimum(np.maximum(e, np.float32(0)), np.float32(1))
    c2 = np.minimum(np.maximum(e, np.float32(0)), np.float32(2))
    s0 = np.asarray(s0, dtype=np.float32)
    s1 = np.asarray(s1, dtype=np.float32)
    r = c1 * s0 + c2 * s1
    if with_acc:
        r = r + in1
    return r.astype(np.float32)


def _register(name: str, spec: Spec) -> DveOp:
    for op in dve_ops_mod.OPS:
        if op.name == name:
            return op
    row = dve_ops_mod._CUSTOM_DVE_ROW_BASE + len(dve_ops_mod.OPS)
    assert row < 0x20, "custom-DVE row overflow"
    sha = {}
    for ver in ("v3", "v4"):
        try:
            s = DveOpSpec(
                name=name,
                opcode=row,
                uops=lower(spec, ver=ver),
                rd1_en=_has_src1(spec),
            )
            sha[ver] = s.sha(ver)
        except Exception:
            pass
    op = DveOp(name, spec, subdim=False, uops_sha=sha)
    dve_ops_mod.OPS.append(op)
    dve_ops_mod.CUSTOM_DVE_SPECS[name] = spec
    dve_ops_mod._SUB_OPCODE_FOR_NAME[name] = row
    return op


AUGLUT_PAIR = _register(
    "AUGLUT_PAIR",
    Spec(
        body=_pair_body(with_acc=True),
        reference=lambda in0, in1, s0, s1, imm2: _np_pair(in0, in1, s0, s1, imm2, True),
    ),
)

AUGLUT_PAIR_INIT = _register(
    "AUGLUT_PAIR_INIT",
    Spec(
        body=_pair_body(with_acc=False),
        reference=lambda in0, in1, s0, s1, imm2: _np_pair(
            in0, None, s0, s1, imm2, False
        ),
    ),
)


# --------------------------------------------------------------------------
# Bass module
# --------------------------------------------------------------------------
def build_module(reps: int = 1):
    """Build the SPMD Bass module. `reps` repeats the whole compute for
    HW-time measurement via wall-clock deltas (output identical)."""
    nc = bacc.Bacc("TRN2", target_bir_lowering=False, debug=False, num_devices=BATCH)

    x_d = nc.dram_tensor("x", [P, FREE], mybir.dt.float32, kind="ExternalInput")
    lut_d = nc.dram_tensor("lut", [P, N_BINS], mybir.dt.float32, kind="ExternalInput")
    o_d = nc.dram_tensor("o", [P, FREE], mybir.dt.float32, kind="ExternalOutput")

    x_ap = x_d.ap()
    o_ap = o_d.ap()

    with TileContext(nc) as tc:
        with (
            tc.tile_pool(name="lutp", bufs=1) as lutp,
            tc.tile_pool(name="work", bufs=3) as wp,
        ):
            lut_t = lutp.tile([P, N_BINS], mybir.dt.float32)
            nc.sync.dma_start(out=lut_t[:], in_=lut_d.ap()[:])
            for _ in range(reps):
                for j in range(N_CHUNKS):
                    sl = slice(j * CHUNK, (j + 1) * CHUNK)
                    xt = wp.tile([P, CHUNK], mybir.dt.float32, tag="x")
                    nc.sync.dma_start(out=xt[:], in_=x_ap[:, sl])
                    tt = wp.tile([P, CHUNK], mybir.dt.float32, tag="t")
                    nc.scalar.mul(out=tt[:], in_=xt[:], mul=19.0)
                    acc = wp.tile([P, CHUNK], mybir.dt.float32, tag="accA")
                    nc.vector._custom_dve(
                        AUGLUT_PAIR_INIT,
                        out=acc[:],
                        in0=tt[:],
                        s0=lut_t[:, 0:1],
                        s1=lut_t[:, 1:2],
                        imm2=-1.0,
                    )
                    for pr in range(1, 10):
                        nxt = wp.tile(
                            [P, CHUNK],
                            mybir.dt.float32,
                            tag="accB" if pr % 2 else "accA",
                        )
                        nc.vector._custom_dve(
                            AUGLUT_PAIR,
                            out=nxt[:],
                            in0=tt[:],
                            in1=acc[:],
                            s0=lut_t[:, 2 * pr : 2 * pr + 1],
                            s1=lut_t[:, 2 * pr + 1 : 2 * pr + 2],
                            imm2=float(2 * pr - 1),
                        )
                        acc = nxt
                    nc.sync.dma_start(out=o_ap[:, sl], in_=acc[:])

    nc.finalize()
    return nc


_MODULE_CACHE: dict[int, object] = {}


def _get_module(reps: int = 1):
    if reps not in _MODULE_CACHE:
        _MODULE_CACHE[reps] = build_module(reps)
    return _MODULE_CACHE[reps]


# --------------------------------------------------------------------------
# Host-side LUT prep
# --------------------------------------------------------------------------
def _make_luts(ran_y: np.ndarray) -> np.ndarray:
    """ran_y [8, 20] -> per-core [8, 128, 20] pair-coefficient tables."""
    y = ran_y.astype(np.float32)
    ymin = y.min(axis=1, keepdims=True)
    ymax = y.max(axis=1, keepdims=True)
    y = (y - ymin) / (ymax - ymin + np.float32(EPS))

    D = np.empty((BATCH, N_BINS), np.float32)
    D[:, 0] = y[:, 0]
    D[:, 1:] = y[:, 1:] - y[:, :-1]

    cols = np.empty((BATCH, N_BINS), np.float32)
    cols[:, 0::2] = D[:, 0::2] - D[:, 1::2]  # s0 of each pair
    cols[:, 1::2] = D[:, 1::2]  # s1 of each pair
    return np.broadcast_to(cols[:, None, :], (BATCH, P, N_BINS)).copy()


# --------------------------------------------------------------------------
# Entry point
# --------------------------------------------------------------------------
def kernel(x: np.ndarray, ran_y: np.ndarray, _reps: int = 1) -> np.ndarray:
    x = np.asarray(x, dtype=np.float32)
    ran_y = np.asarray(ran_y, dtype=np.float32)
    assert x.shape == (BATCH, *SPATIAL), x.shape
    assert ran_y.shape == (BATCH, N_BINS), ran_y.shape

    nc = _get_module(_reps)
    luts = _make_luts(ran_y)
    xr = np.ascontiguousarray(x.reshape(BATCH, P, FREE))
    in_maps = [{"x": xr[b], "lut": luts[b]} for b in range(BATCH)]

    res = bass_utils.run_bass_kernel_spmd(nc, in_maps, core_ids=list(range(BATCH)))
    out = np.stack([res.results[b]["o"] for b in range(BATCH)], axis=0)
    return out.reshape(BATCH, *SPATIAL)


# revision 12
# speedup vs baseline: 342.0658x; 342.0658x over previous
"""AugLUT Trainium2 kernel: per-batch random 20-knot LUT applied to x via
piecewise-linear interpolation.

Math: with t = 19*x and the per-batch normalized LUT y[0..19],
    f(t) = sum_{k=-1}^{18} D_k * clamp(t - k, 0, 1)
where D_{-1} = y_0 (clamp == 1 for all t >= 0) and D_k = y_{k+1} - y_k.
Pairs of consecutive terms fuse into ONE custom DVE instruction via
    clamp(e-1,0,1) = clamp(e,0,2) - clamp(e,0,1):
    pair(a) = (D_a - D_{a+1})*clamp(e,0,1) + D_{a+1}*clamp(e,0,2),  e = t - a
so the whole 20-term LUT costs 10 DVE instructions per tile (+1 ACT scale).

Sharding: pure data parallel — batch b -> NeuronCore b (8 cores), the tiny
LUT rides along as a [128, 20] per-partition-broadcast input per core.
"""

import sys

if "/opt/trn_rl_repo" not in sys.path:
    sys.path.insert(0, "/opt/trn_rl_repo")

import numpy as np

import concourse.bacc as bacc
import concourse.dve_ops as dve_ops_mod
import concourse.mybir as mybir
from concourse import bass_utils
from concourse.dve_ops import DveOp
from concourse.dve_spec import (
    C0,
    C1,
    C2,
    Latch,
    One,
    Spec,
    Src0,
    Src1,
    Zero,
    lower,
    maxx,
    minn,
    _has_src1,
)
from concourse.dve_uop import DveOpSpec
from concourse.tile import TileContext

N_BINS = 20
EPS = 1e-5
BATCH = 8
SPATIAL = (192, 192, 192)
N_ELEM = 192 * 192 * 192  # 7_077_888
P = 128
FREE = N_ELEM // P  # 55296
CHUNK = 2048
N_CHUNKS = FREE // CHUNK  # 27


# --------------------------------------------------------------------------
# Custom DVE op registration (runtime, self-signed sha)
# --------------------------------------------------------------------------
def _pair_body(with_acc: bool):
    e = Src0 - C2
    r = maxx(e, Zero)
    c1 = minn(r, One)
    p1 = c1 * C0
    c2 = minn(r, One + One)
    if with_acc:
        a = Src1 + p1
        p2 = c2 * Latch(maxx(C1, C1))
        return a + p2
    p2 = c2 * C1
    return p1 + p2


def _np_pair(in0, in1, s0, s1, imm2, with_acc):
    e = in0.astype(np.float32) - np.float32(imm2)
    c1 = np.minimum(np.maximum(e, np.float32(0)), np.float32(1))
    c2 = np.minimum(np.maximum(e, np.float32(0)), np.float32(2))
    s0 = np.asarray(s0, dtype=np.float32)
    s1 = np.asarray(s1, dtype=np.float32)
    r = c1 * s0 + c2 * s1
    if with_acc:
        r = r + in1
    return r.astype(np.float32)


def _register(name: str, spec: Spec) -> DveOp:
    for op in dve_ops_mod.OPS:
        if op.name == name:
            return op
    row = dve_ops_mod._CUSTOM_DVE_ROW_BASE + len(dve_ops_mod.OPS)
    assert row < 0x20, "custom-DVE row overflow"
    sha = {}
    for ver in ("v3", "v4"):
        try:
            s = DveOpSpec(
                name=name,
                opcode=row,
                uops=lower(spec, ver=ver),
                rd1_en=_has_src1(spec),
            )
            sha[ver] = s.sha(ver)
        except Exception:
            pass
    op = DveOp(name, spec, subdim=False, uops_sha=sha)
    dve_ops_mod.OPS.append(op)
    dve_ops_mod.CUSTOM_DVE_SPECS[name] = spec
    dve_ops_mod._SUB_OPCODE_FOR_NAME[name] = row
    return op


AUGLUT_PAIR = _register(
    "AUGLUT_PAIR",
    Spec(
        body=_pair_body(with_acc=True),
        reference=lambda in0, in1, s0, s1, imm2: _np_pair(in0, in1, s0, s1, imm2, True),
    ),
)

AUGLUT_PAIR_INIT = _register(
    "AUGLUT_PAIR_INIT",
    Spec(
        body=_pair_body(with_acc=False),
        reference=lambda in0, in1, s0, s1, imm2: _np_pair(
            in0, None, s0, s1, imm2, False
        ),
    ),
)


# --------------------------------------------------------------------------
# Bass module
# --------------------------------------------------------------------------
def _act_chunk_set(n_chunks: int, k: int, layout: str = "head") -> set:
    if k <= 0:
        return set()
    if layout == "head":
        return set(range(k))
    return {int(round(i * n_chunks / k)) % n_chunks for i in range(k)}


def build_module(
    reps: int = 1,
    chunk: int = CHUNK,
    bufs: int = 3,
    inplace: bool = True,
    act_k: int = 0,
    act_layout: str = "head",
    copy_eng: str = "act",
):
    """Build the SPMD Bass module.

    `reps` repeats the whole compute (HW-time measurement via deltas).
    `act_k` chunks (of FREE//chunk) are routed to the ACT+PE pipeline
    (relu-basis: 19 ACT ops + PSUM accumulate with runtime diag weights);
    the rest use the DVE clamp-pair chain.
    """
    nc = bacc.Bacc("TRN2", target_bir_lowering=False, debug=False, num_devices=BATCH)

    f32 = mybir.dt.float32
    x_d = nc.dram_tensor("x", [P, FREE], f32, kind="ExternalInput")
    lut_d = nc.dram_tensor("lut", [P, N_BINS], f32, kind="ExternalInput")
    if act_k > 0:
        wts_d = nc.dram_tensor("wts", [P, 19 * P], f32, kind="ExternalInput")
        ab_d = nc.dram_tensor("ab", [P, 20], f32, kind="ExternalInput")
    o_d = nc.dram_tensor("o", [P, FREE], f32, kind="ExternalOutput")

    x_ap = x_d.ap()
    o_ap = o_d.ap()
    n_chunks = FREE // chunk
    assert n_chunks * chunk == FREE, (chunk, FREE)
    act_set = _act_chunk_set(n_chunks, act_k, act_layout)
    n_sub = chunk // 512

    with TileContext(nc) as tc:
        with (
            tc.tile_pool(name="lutp", bufs=1) as lutp,
            tc.tile_pool(name="work", bufs=bufs) as wp,
            tc.tile_pool(name="worka", bufs=max(2, bufs - 1)) as wpa,
            tc.tile_pool(name="psum", bufs=2, space="PSUM") as pp,
        ):
            lut_t = lutp.tile([P, N_BINS], f32)
            nc.sync.dma_start(out=lut_t[:], in_=lut_d.ap()[:])
            if act_k > 0:
                wts_t = lutp.tile([P, 19 * P], f32)
                nc.sync.dma_start(out=wts_t[:], in_=wts_d.ap()[:])
                ab_t = lutp.tile([P, 20], f32)
                nc.sync.dma_start(out=ab_t[:], in_=ab_d.ap()[:])

            def dve_chunk(sl):
                xt = wp.tile([P, chunk], f32, tag="x")
                nc.sync.dma_start(out=xt[:], in_=x_ap[:, sl])
                if inplace:
                    tt = xt
                else:
                    tt = wp.tile([P, chunk], f32, tag="t")
                nc.scalar.mul(out=tt[:], in_=xt[:], mul=19.0)
                acc = wp.tile([P, chunk], f32, tag="accA")
                nc.vector._custom_dve(
                    AUGLUT_PAIR_INIT,
                    out=acc[:],
                    in0=tt[:],
                    s0=lut_t[:, 0:1],
                    s1=lut_t[:, 1:2],
                    imm2=-1.0,
                )
                for pr in range(1, 10):
                    nxt = (
                        acc
                        if inplace
                        else wp.tile([P, chunk], f32, tag="accB" if pr % 2 else "accA")
                    )
                    nc.vector._custom_dve(
                        AUGLUT_PAIR,
                        out=nxt[:],
                        in0=tt[:],
                        in1=acc[:],
                        s0=lut_t[:, 2 * pr : 2 * pr + 1],
                        s1=lut_t[:, 2 * pr + 1 : 2 * pr + 2],
                        imm2=float(2 * pr - 1),
                    )
                    acc = nxt
                nc.sync.dma_start(out=o_ap[:, sl], in_=acc[:])

            def act_chunk(sl):
                xa = wpa.tile([P, chunk], f32, tag="xa")
                nc.sync.dma_start(out=xa[:], in_=x_ap[:, sl])
                ps = pp.tile([P, chunk], f32)
                for j in range(19):
                    r = wpa.tile([P, chunk], f32, tag="r")
                    if j == 0:
                        # affine term: A + 19B*x
                        nc.scalar.activation(
                            out=r[:],
                            in_=xa[:],
                            func=mybir.ActivationFunctionType.Identity,
                            bias=ab_t[:, 0:1],
                            scale=ab_t[:, 1:2],
                        )
                    else:
                        # relu basis: relu(19x - j), coefficient rides PE weights
                        nc.scalar.activation(
                            out=r[:],
                            in_=xa[:],
                            func=mybir.ActivationFunctionType.Relu,
                            bias=ab_t[:, 1 + j : 2 + j],
                            scale=19.0,
                        )
                    w_sl = wts_t[:, j * P : (j + 1) * P]
                    for i in range(n_sub):
                        ss = slice(i * 512, (i + 1) * 512)
                        nc.tensor.matmul(
                            ps[:, ss],
                            w_sl,
                            r[:, ss],
                            start=(j == 0),
                            stop=(j == 18),
                        )
                os_t = wpa.tile([P, chunk], f32, tag="os")
                if copy_eng == "act":
                    nc.scalar.copy(out=os_t[:], in_=ps[:])
                else:
                    nc.vector.tensor_copy(out=os_t[:], in_=ps[:])
                nc.sync.dma_start(out=o_ap[:, sl], in_=os_t[:])

            def body():
                for j in range(n_chunks):
                    sl = slice(j * chunk, (j + 1) * chunk)
                    if j in act_set:
                        act_chunk(sl)
                    else:
                        dve_chunk(sl)

            if reps == 1:
                body()
            else:
                with tc.For_i(
                    0,
                    reps,
                    1,
                    hint_engines=(
                        mybir.EngineType.DVE,
                        mybir.EngineType.SP,
                        mybir.EngineType.Activation,
                        mybir.EngineType.PE,
                    ),
                ):
                    body()

    nc.finalize()
    return nc


_MODULE_CACHE: dict[tuple, object] = {}


def _get_module(reps: int = 1, **cfg):
    key = (reps, tuple(sorted(cfg.items())))
    if key not in _MODULE_CACHE:
        _MODULE_CACHE[key] = build_module(reps, **cfg)
    return _MODULE_CACHE[key]


# --------------------------------------------------------------------------
# Host-side LUT prep
# --------------------------------------------------------------------------
def _make_luts(ran_y: np.ndarray):
    """ran_y [8, 20] -> (lut [8,128,20], wts [8,128,19*128], ab [8,128,2])."""
    y = ran_y.astype(np.float32)
    ymin = y.min(axis=1, keepdims=True)
    ymax = y.max(axis=1, keepdims=True)
    y = (y - ymin) / (ymax - ymin + np.float32(EPS))

    D = np.empty((BATCH, N_BINS), np.float32)
    D[:, 0] = y[:, 0]
    D[:, 1:] = y[:, 1:] - y[:, :-1]

    cols = np.empty((BATCH, N_BINS), np.float32)
    cols[:, 0::2] = D[:, 0::2] - D[:, 1::2]  # s0 of each pair
    cols[:, 1::2] = D[:, 1::2]  # s1 of each pair
    lut = np.broadcast_to(cols[:, None, :], (BATCH, P, N_BINS)).copy()

    # relu-basis for the ACT+PE path:
    # f(t) = A + B*t + sum_{j=1..18} c_j*relu(t-j);  t = 19x
    A = y[:, 0]  # [8]
    B = y[:, 1] - y[:, 0]
    c = (y[:, 2:] - y[:, 1:-1]) - (y[:, 1:-1] - y[:, :-2])  # [8, 18]
    wts = np.zeros((BATCH, P, 19 * P), np.float32)
    di = np.arange(P)
    wts[:, di, di] = 1.0  # term 0: identity
    for j in range(1, 19):
        wts[:, di, j * P + di] = c[:, j - 1][:, None]
    ab = np.empty((BATCH, P, 20), np.float32)
    ab[:, :, 0] = A[:, None]
    ab[:, :, 1] = (np.float32(19.0) * B)[:, None]
    ab[:, :, 2:] = -np.arange(1, 19, dtype=np.float32)[None, None, :]
    return lut, wts, ab


# --------------------------------------------------------------------------
# Entry point
# --------------------------------------------------------------------------
ACT_K = 6  # chunks routed to the ACT+PE pipeline (of FREE//CHUNK)


def kernel(x: np.ndarray, ran_y: np.ndarray, _reps: int = 1, **_cfg) -> np.ndarray:
    x = np.asarray(x, dtype=np.float32)
    ran_y = np.asarray(ran_y, dtype=np.float32)
    assert x.shape == (BATCH, *SPATIAL), x.shape
    assert ran_y.shape == (BATCH, N_BINS), ran_y.shape

    cfg = {"act_k": ACT_K, "bufs": 4, **_cfg}
    nc = _get_module(_reps, **cfg)
    lut, wts, ab = _make_luts(ran_y)
    xr = np.ascontiguousarray(x.reshape(BATCH, P, FREE))
    in_maps = []
    for b in range(BATCH):
        m = {"x": xr[b], "lut": lut[b]}
        if cfg.get("act_k", 0) > 0:
            m["wts"] = wts[b]
            m["ab"] = ab[b]
        in_maps.append(m)

    res = bass_utils.run_bass_kernel_spmd(nc, in_maps, core_ids=list(range(BATCH)))
    out = np.stack([res.results[b]["o"] for b in range(BATCH)], axis=0)
    return out.reshape(BATCH, *SPATIAL)
